# revision 7
# baseline (speedup 1.0000x reference)
"""Trainium2 Bass kernel for nn_Lowpass: per-128-block RBJ lowpass biquad.

Algorithm (per 128-sample block, zero initial state):
  y = IIR(FIR(x)) with per-block coefficients from avg-pooled control params.
  FIR: u[n] = x[n] + 2 x[n-1] + x[n-2]    (b0 factored out; b2 == b0, b1 == 2 b0)
  IIR poles are complex (r e^{+-i theta}).  Rotated-frame decomposition turns
  the order-2 recurrence into two real first-order scans that map directly to
  the DVE tensor_tensor_scan instruction:
      v_re[n] = r v_re[n-1] + cos(n theta) u[n]
      v_im[n] = r v_im[n-1] - sin(n theta) u[n]
      y[n]    = Z b0 (cos(n theta + phi) v_re[n] - sin(n theta + phi) v_im[n])
  with 2c = 1 - i pr/pi the pole residue, Z = |2c|, phi = arg(2c).
  (Here the kernel scans d_im = +sin * u, flipping the recombine sign.)

Sharding: pure data parallel, core c processes batches [4c, 4c+4).
"""

import sys

sys.path.insert(0, "/opt/trn_rl_repo")

import math

import numpy as np

import concourse.bacc as bacc
import concourse.bass as bass
import concourse.mybir as mybir
from concourse.tile import TileContext

F32 = mybir.dt.float32
AX = mybir.AxisListType
ALU = mybir.AluOpType
ACT = mybir.ActivationFunctionType

SR = 44100.0
BLOCK = 128
FC_MIN, FC_MAX = 2000.0, 20000.0
Q_MIN, Q_MAX = 0.1, 10.0
PI = math.pi


def build_core_kernel(NB=4, S=262144, n_devices=8):
    """Bass kernel for one core: NB batches of S samples."""
    P = 128
    F = S // P            # free elems per row (per batch)
    HI = F // BLOCK       # blocks per partition row
    nblk = S // BLOCK     # blocks per batch

    nc = bacc.Bacc("TRN2", target_bir_lowering=False, debug=False,
                   num_devices=n_devices)
    x_d = nc.dram_tensor("x", [NB, S], F32, kind="ExternalInput")
    cp_d = nc.dram_tensor("cp", [NB, 2, S], F32, kind="ExternalInput")
    y_d = nc.dram_tensor("y", [NB, S], F32, kind="ExternalOutput")
    fc_d = nc.dram_tensor("fc", [NB, nblk], F32, kind="ExternalOutput")
    q_d = nc.dram_tensor("q", [NB, nblk], F32, kind="ExternalOutput")

    NBH = NB * HI  # total coeff lanes per partition

    with TileContext(nc) as tc:
        from contextlib import ExitStack
        with ExitStack() as ctx:
            cpool = ctx.enter_context(tc.tile_pool(name="const", bufs=1))
            spool = ctx.enter_context(tc.tile_pool(name="small", bufs=1))
            cppool = ctx.enter_context(tc.tile_pool(name="cpstage", bufs=3))
            big = ctx.enter_context(tc.tile_pool(name="big", bufs=2))

            # ---- constants ----
            iota_i = cpool.tile([P, BLOCK], mybir.dt.int32, tag="iota_i")
            nc.gpsimd.iota(iota_i[:], pattern=[[1, BLOCK]], base=0,
                           channel_multiplier=0)
            iota_f = cpool.tile([P, BLOCK], F32, tag="iota_f")
            nc.vector.tensor_copy(iota_f[:], iota_i[:])

            _consts = {}

            def c_ap(val):
                if val not in _consts:
                    t = cpool.tile([P, 1], F32, tag=f"c{len(_consts)}",
                                   name=f"c{len(_consts)}")
                    nc.vector.memset(t[:], val)
                    _consts[val] = t
                return _consts[val][:]

            # ---- stage 1: pool control params, coefficient math ----
            sum0 = spool.tile([P, NBH], F32, tag="sum0")
            sum1 = spool.tile([P, NBH], F32, tag="sum1")
            for b in range(NB):
                for prm in range(2):
                    cpt = cppool.tile([P, F], F32, tag="cpt")
                    nc.sync.dma_start(
                        out=cpt[:], in_=cp_d[b, prm].rearrange("(p f) -> p f", p=P))
                    dst = (sum0 if prm == 0 else sum1)[:, b * HI:(b + 1) * HI]
                    nc.vector.tensor_reduce(
                        dst, cpt[:].rearrange("p (h t) -> p h t", t=BLOCK),
                        axis=AX.X, op=ALU.add)

            def stile(tag):
                return spool.tile([P, NBH], F32, tag=tag, name=tag)

            # fc, q (also outputs)
            fc = stile("fc")
            nc.vector.tensor_scalar(fc[:], sum0[:], (FC_MAX - FC_MIN) / BLOCK,
                                    FC_MIN, op0=ALU.mult, op1=ALU.add)
            q = stile("q")
            nc.vector.tensor_scalar(q[:], sum1[:], (Q_MAX - Q_MIN) / BLOCK,
                                    Q_MIN, op0=ALU.mult, op1=ALU.add)
            nc.sync.dma_start(
                out=fc_d.rearrange("b (p h) -> p b h", p=P),
                in_=fc[:].rearrange("p (b h) -> p b h", b=NB))
            nc.sync.dma_start(
                out=q_d.rearrange("b (p h) -> p b h", p=P),
                in_=q[:].rearrange("p (b h) -> p b h", b=NB))

            # w0 = 2 pi fc / SR
            w0 = stile("w0")
            nc.vector.tensor_scalar(
                w0[:], sum0[:], (FC_MAX - FC_MIN) / BLOCK * 2.0 * PI / SR,
                FC_MIN * 2.0 * PI / SR, op0=ALU.mult, op1=ALU.add)
            sinw = stile("sinw")
            nc.scalar.activation(sinw[:], w0[:], ACT.Sin)
            cosw = stile("cosw")
            nc.scalar.activation(cosw[:], w0[:], ACT.Sin, scale=-1.0, bias=c_ap(PI / 2))

            # alpha = sinw / (2 q);  a0r = 1/(1+alpha)
            qr = stile("qr")
            nc.vector.reciprocal(qr[:], q[:])
            alpha = stile("alpha")
            nc.vector.scalar_tensor_tensor(alpha[:], sinw[:], 0.5, qr[:],
                                           op0=ALU.mult, op1=ALU.mult)
            t0 = stile("t0")
            nc.vector.tensor_scalar_add(t0[:], alpha[:], 1.0)
            a0r = stile("a0r")
            nc.vector.reciprocal(a0r[:], t0[:])

            # b0 = (1-cosw)/2 * a0r ; pr = cosw * a0r ; a2 = (1-alpha) * a0r
            t1 = stile("t1")
            nc.vector.tensor_scalar(t1[:], cosw[:], -0.5, 0.5,
                                    op0=ALU.mult, op1=ALU.add)
            b0 = stile("b0")
            nc.vector.tensor_tensor(b0[:], t1[:], a0r[:], op=ALU.mult)
            pr = stile("pr")
            nc.vector.tensor_tensor(pr[:], cosw[:], a0r[:], op=ALU.mult)
            t2 = stile("t2")
            nc.vector.tensor_scalar(t2[:], alpha[:], -1.0, 1.0,
                                    op0=ALU.mult, op1=ALU.add)
            a2 = stile("a2")
            nc.vector.tensor_tensor(a2[:], t2[:], a0r[:], op=ALU.mult)

            # r = sqrt(a2) = exp(0.5 ln a2); pi2 = a2 - pr^2; pi_ = sqrt(pi2)
            ln_a2 = stile("ln_a2")
            nc.scalar.activation(ln_a2[:], a2[:], ACT.Ln)
            r_t = stile("r_t")
            nc.scalar.activation(r_t[:], ln_a2[:], ACT.Exp, scale=0.5)
            prsq = stile("prsq")
            nc.vector.tensor_tensor(prsq[:], pr[:], pr[:], op=ALU.mult)
            pi2 = stile("pi2")
            nc.vector.tensor_tensor(pi2[:], a2[:], prsq[:], op=ALU.subtract)
            nc.vector.tensor_scalar_max(pi2[:], pi2[:], 1e-12)
            ln_p = stile("ln_p")
            nc.scalar.activation(ln_p[:], pi2[:], ACT.Ln)
            pi_ = stile("pi_")
            nc.scalar.activation(pi_[:], ln_p[:], ACT.Exp, scale=0.5)

            # ratio = pr/pi_ ; theta = pi/2 - atan(ratio) ; phi = -atan(ratio)
            pir = stile("pir")
            nc.vector.reciprocal(pir[:], pi_[:])
            ratio = stile("ratio")
            nc.vector.tensor_tensor(ratio[:], pr[:], pir[:], op=ALU.mult)
            atn = stile("atn")  # = -atan(ratio) = phi
            nc.scalar.activation(atn[:], ratio[:], ACT.Arctan, scale=-1.0)
            theta = stile("theta")
            nc.vector.tensor_scalar_add(theta[:], atn[:], PI / 2)

            # Z = sqrt(1 + ratio^2) ; ZB = Z * b0
            rsq1 = stile("rsq1")
            nc.vector.scalar_tensor_tensor(rsq1[:], ratio[:], 1.0, ratio[:],
                                           op0=ALU.mult, op1=ALU.mult)
            nc.vector.tensor_scalar_add(rsq1[:], rsq1[:], 1.0)
            ln_z = stile("ln_z")
            nc.scalar.activation(ln_z[:], rsq1[:], ACT.Ln)
            z_t = stile("z_t")
            nc.scalar.activation(z_t[:], ln_z[:], ACT.Exp, scale=0.5)
            zb = stile("zb")
            nc.vector.tensor_tensor(zb[:], z_t[:], b0[:], op=ALU.mult)

            # ---- stage 2: per-batch streaming filter ----
            _bufs2 = {"x", "ang", "mtmp", "mred"}

            def bt(tag):
                return big.tile([P, F], F32, tag=tag, name=tag,
                                bufs=2 if tag in _bufs2 else 1)

            for b in range(NB):
                sl = slice(b * HI, (b + 1) * HI)

                x_sb = bt("x")
                nc.sync.dma_start(
                    out=x_sb[:], in_=x_d[b].rearrange("(p f) -> p f", p=P))
                xv = x_sb[:].rearrange("p (h t) -> p h t", t=BLOCK)

                # FIR u = x + 2 x_{-1} + x_{-2} (per block; fix cols 0,1)
                u1 = bt("u1")
                nc.vector.scalar_tensor_tensor(
                    u1[:, 1:], x_sb[:, :F - 1], 2.0, x_sb[:, 1:],
                    op0=ALU.mult, op1=ALU.add)
                nc.vector.tensor_copy(u1[:, 0:1], x_sb[:, 0:1])
                u = bt("u")
                nc.vector.tensor_tensor(u[:, 2:], u1[:, 2:], x_sb[:, :F - 2],
                                        op=ALU.add)
                uv = u[:].rearrange("p (h t) -> p h t", t=BLOCK)
                nc.vector.tensor_copy(uv[:, :, 0:1], xv[:, :, 0:1])
                nc.vector.scalar_tensor_tensor(
                    uv[:, :, 1:2], xv[:, :, 0:1], 2.0, xv[:, :, 1:2],
                    op0=ALU.mult, op1=ALU.add)

                # angle grids
                iota_b = iota_f[:].unsqueeze(1).broadcast_to(
                    (P, HI, BLOCK))
                th_b = theta[:, sl].unsqueeze(2).broadcast_to(
                    (P, HI, BLOCK))
                ang = bt("ang")
                angv = ang[:].rearrange("p (h t) -> p h t", t=BLOCK)
                nc.vector.tensor_tensor(angv, iota_b, th_b, op=ALU.mult)
                phi_b = atn[:, sl].unsqueeze(2).broadcast_to(
                    (P, HI, BLOCK))
                psi = bt("psi")
                psiv = psi[:].rearrange("p (h t) -> p h t", t=BLOCK)
                nc.vector.tensor_tensor(psiv, angv, phi_b, op=ALU.add)

                MAGIC = 1.5 * 2.0 ** 23  # fp32 round-to-nearest-int bias
                INV2PI = 1.0 / (2.0 * PI)
                TWOPI = 2.0 * PI

                def trig(src, ctag, stag):
                    # m = src - 2pi*round(src/2pi) in [-pi, pi]
                    t = bt("mtmp")
                    nc.vector.tensor_scalar(t[:], src[:], INV2PI, MAGIC,
                                            op0=ALU.mult, op1=ALU.add)
                    nc.vector.tensor_scalar_sub(t[:], t[:], MAGIC)
                    m = bt("mred")
                    nc.vector.scalar_tensor_tensor(m[:], t[:], -TWOPI, src[:],
                                                   op0=ALU.mult, op1=ALU.add)
                    nc.vector.tensor_scalar(m[:], m[:], -PI, PI,
                                            op0=ALU.max, op1=ALU.min)
                    # sin(src) = sin(m);  cos(src) = sin(pi/2 - |m|)
                    sgr = bt(stag)
                    nc.scalar.activation(sgr[:], m[:], ACT.Sin)
                    nc.scalar.activation(m[:], m[:], ACT.Abs)
                    cgr = bt(ctag)
                    nc.scalar.activation(cgr[:], m[:], ACT.Sin, scale=-1.0,
                                         bias=c_ap(PI / 2))
                    return cgr, sgr

                cg, sg = trig(ang, "cg", "sg")     # cos(ang), sin(ang)
                cpg, spg = trig(psi, "cpg", "spg")  # cos(psi), sin(psi)

                # scan multiplier grid: r per lane, 0 at block starts
                d0 = bt("d0")
                d0v = d0[:].rearrange("p (h t) -> p h t", t=BLOCK)
                r_b = r_t[:, sl].unsqueeze(2).broadcast_to(
                    (P, HI, BLOCK))
                nc.vector.tensor_copy(d0v, r_b)
                nc.vector.memset(d0v[:, :, 0:1], 0.0)

                dre = bt("dre")
                nc.vector.tensor_tensor(dre[:], cg[:], u[:], op=ALU.mult)
                dim = bt("dim")
                nc.vector.tensor_tensor(dim[:], sg[:], u[:], op=ALU.mult)

                vre = bt("vre")
                nc.vector.tensor_tensor_scan(vre[:], d0[:], dre[:], 0.0,
                                             op0=ALU.mult, op1=ALU.add)
                vim = bt("vim")
                nc.vector.tensor_tensor_scan(vim[:], d0[:], dim[:], 0.0,
                                             op0=ALU.mult, op1=ALU.add)

                # y = ZB * (cos(psi) v_re + sin(psi) v_im')   [v_im' = -v_im]
                m1 = bt("dre")
                nc.vector.tensor_tensor(m1[:], cpg[:], vre[:], op=ALU.mult)
                m2 = bt("dim")
                nc.vector.tensor_tensor(m2[:], spg[:], vim[:], op=ALU.mult)
                s = bt("u")
                nc.vector.tensor_tensor(s[:], m1[:], m2[:], op=ALU.add)
                y = bt("ang")
                zb_b = zb[:, sl].unsqueeze(2).broadcast_to(
                    (P, HI, BLOCK))
                yv = y[:].rearrange("p (h t) -> p h t", t=BLOCK)
                nc.vector.tensor_tensor(yv, s[:].rearrange(
                    "p (h t) -> p h t", t=BLOCK), zb_b, op=ALU.mult)

                nc.sync.dma_start(
                    out=y_d[b].rearrange("(p f) -> p f", p=P), in_=y[:])

    nc.compile()
    return nc


_NC_CACHE = {}


def _get_nc(NB, S):
    key = (NB, S)
    if key not in _NC_CACHE:
        _NC_CACHE[key] = build_core_kernel(NB, S)
    return _NC_CACHE[key]


def kernel(x: np.ndarray, control_params: np.ndarray):
    """Full-input entry: x (32,1,262144), control_params (32,2,262144).
    Returns (out, fc, q) matching reference."""
    from concourse.bass_utils import run_bass_kernel_spmd

    B, _, S = x.shape
    n_cores = 8
    nb = B // n_cores
    nblk = S // BLOCK
    nc = _get_nc(nb, S)

    x2 = np.ascontiguousarray(x[:, 0, :], dtype=np.float32)
    cp = np.ascontiguousarray(control_params, dtype=np.float32)
    in_maps = [
        {"x": x2[c * nb:(c + 1) * nb], "cp": cp[c * nb:(c + 1) * nb]}
        for c in range(n_cores)
    ]
    res = run_bass_kernel_spmd(nc, in_maps, list(range(n_cores)))

    out = np.empty((B, 1, S), dtype=np.float32)
    fc = np.empty((B, nblk), dtype=np.float32)
    q = np.empty((B, nblk), dtype=np.float32)
    for c in range(n_cores):
        rd = res.results[c]
        out[c * nb:(c + 1) * nb, 0, :] = rd["y"]
        fc[c * nb:(c + 1) * nb] = rd["fc"]
        q[c * nb:(c + 1) * nb] = rd["q"]
    return out, fc, q
